# revision 1
# baseline (speedup 1.0000x reference)
"""MoE (top-2 of 8 experts) Trainium2 kernel.

Strategy: expert-parallel across 8 NeuronCores, one expert per core.
Host computes the (tiny) router + top-2 token dispatch; each core runs
the heavy expert FFN (x @ w1 -> gelu -> @ w2) over only the tokens
routed to its expert (~T*K/E tokens), in bf16 with fp32 accumulation.
Host applies the renormalized top-2 gates + b2 and scatter-adds the
per-expert outputs back into the full [B,S,D] output.

Shapes (hardcoded from the problem spec): B=4, S=2048, D=768, E=8,
F=4*D=3072, TOP_K=2.
"""

import os
import sys
import types

import numpy as np
import ml_dtypes

# concourse.bass_utils imports antenv.axon_hooks when tracing is requested
# (e.g. BASS_TRACE=1); some deployments lack that module. Provide a stub so
# tracing degrades gracefully (run without trace) instead of crashing.
try:
    from antenv import axon_hooks as _axon_hooks  # noqa: F401
except ImportError:
    _m = types.ModuleType("antenv.axon_hooks")
    _m._hook = None
    _m.set_axon_ntff_profile_hook = lambda h: setattr(_m, "_hook", h)
    _m.get_axon_ntff_profile_hook = lambda: _m._hook
    sys.modules["antenv.axon_hooks"] = _m
    try:
        import antenv

        antenv.axon_hooks = _m
    except ImportError:
        pass

import concourse.bass as bass
import concourse.tile as tile
from concourse import bacc, mybir
from concourse.bass_utils import run_bass_kernel_spmd

P = 128
D = 768
F = 3072
E = 8
TOP_K = 2
N_CORES = 8

bf16 = mybir.dt.bfloat16
f32 = mybir.dt.float32

# Stash of the most recent BassKernelResults (for test harness introspection).
last_results = None


def _chunks_of(total, size):
    """Split into chunks of `size`, avoiding a tail chunk under 256 when
    possible (small-N matmuls pay proportionally more issue overhead)."""
    out = []
    t0 = 0
    while t0 < total:
        rem = total - t0
        if size < rem < size + 256 and rem - 256 >= 256:
            out.append((t0, rem - 256))
            out.append((t0 + rem - 256, 256))
            break
        cs = min(size, rem)
        out.append((t0, cs))
        t0 += cs
    return out


def _build(C):
    """Expert FFN kernel: yT[D, C] = (gelu(x @ w1 + b1) @ w2).T

    Inputs arrive pre-transposed / pre-permuted so every contraction dim
    lands on SBUF partitions without any on-device transpose:
      xT[D, C]          x transposed
      w1p[nF, P, nD*P]  w1p[fi, p, d*P + c] = w1[d*P + p, fi*P + c]
      w2[F, D]          natural layout
      b1p[P, nF]        b1p[p, o] = b1[o*P + p]
    Output is yT[dout, tok] (host transposes back).
    Per-token gates and b2 are intentionally NOT applied here (host does
    that); this keeps every device instruction to <=1 sync wait.
    """
    nD, nF, nT = D // P, F // P, C // P
    nc = bacc.Bacc(
        "TRN2", target_bir_lowering=False, debug=False, num_devices=N_CORES
    )
    xT = nc.declare_dram_parameter("xT", [D, C], bf16, isOutput=False)
    w1p = nc.declare_dram_parameter("w1p", [nF, P, nD * P], bf16, isOutput=False)
    w2 = nc.declare_dram_parameter("w2", [F, D], bf16, isOutput=False)
    b1p = nc.declare_dram_parameter("b1p", [P, nF], f32, isOutput=False)
    yT = nc.declare_dram_parameter("yT", [D, C], f32, isOutput=True)

    TOK_CHUNK = 512
    chunks = _chunks_of(C, TOK_CHUNK)

    with tile.TileContext(nc) as tc:
        with (
            tc.tile_pool(name="const", bufs=1) as const_pool,
            tc.tile_pool(name="hpool", bufs=2) as hpool,
            tc.tile_pool(name="psum1", bufs=4, space="PSUM") as psum1,
            tc.tile_pool(name="psum2", bufs=3, space="PSUM") as psum2,
            tc.tile_pool(name="outp", bufs=6) as outp,
        ):
            # Grouped input DMAs (few large transfers; ~0.6 us issue cost
            # each on the Sync queue), ordered so chunk-0 compute can start
            # after a ~1.6 MB prefix (inputs are HBM-BW bound, ~13 MB).
            xT_r = xT.rearrange("(o p) t -> p o t", p=P)
            x_sb = []
            t0, cs = chunks[0]
            xt = const_pool.tile([P, nD, TOK_CHUNK], bf16, tag="x_0")
            nc.sync.dma_start(xt[:, :, :cs], xT_r[:, :, t0 : t0 + cs])
            x_sb.append(xt)

            W1G = 2  # f-tiles per w1 DMA group
            w1_sb = []
            for g in range(nF // W1G):
                t = const_pool.tile([P, W1G, nD * P], bf16, tag=f"w1g_{g}")
                nc.sync.dma_start(
                    t[:], w1p[g * W1G : (g + 1) * W1G].rearrange("f p dc -> p f dc")
                )
                w1_sb.append(t)
                if g == 0:
                    b1_sb = const_pool.tile([P, nF], f32)
                    nc.sync.dma_start(b1_sb[:], b1p[:, :])
                    # Pre-touch b1 on the scalar engine so the gelu
                    # activations (which carry the bias as a pointer operand
                    # and thus have only one sync-wait slot) never need to
                    # wait on the DMA.
                    scratch = const_pool.tile([P, 1], f32)
                    nc.scalar.copy(scratch[:], b1_sb[:, 0:1])

            W2G = 4  # f-tiles per w2 DMA group
            w2_sb = []
            for g in range(nF // W2G):
                t = const_pool.tile([P, W2G, D], bf16, tag=f"w2g_{g}")
                nc.sync.dma_start(
                    t[:],
                    w2[g * W2G * P : (g + 1) * W2G * P, :].rearrange(
                        "(f p) d -> p f d", p=P
                    ),
                )
                w2_sb.append(t)

            for ci in range(1, len(chunks)):
                t0, cs = chunks[ci]
                xt = const_pool.tile([P, nD, TOK_CHUNK], bf16, tag=f"x_{ci}")
                nc.sync.dma_start(xt[:, :, :cs], xT_r[:, :, t0 : t0 + cs])
                x_sb.append(xt)

            def w1_tile(fi, d):
                return w1_sb[fi // W1G][:, fi % W1G, d * P : (d + 1) * P]

            def w2_tile(fi, do):
                return w2_sb[fi // W2G][:, fi % W2G, do * P : (do + 1) * P]

            # Dummy matmuls on a zeroed tile while input DMAs stream in:
            # keeps the PE busy through the HAM activity window so the
            # real matmuls start at 2.4 GHz instead of the cold 1.2 GHz.
            warm_src = const_pool.tile([P, P], bf16)
            nc.any.memset(warm_src[:], 0.0)
            for _w in range(33):
                pw = psum1.tile([P, TOK_CHUNK], f32, tag="ph", name="pw")
                for k in range(4):
                    nc.tensor.matmul(
                        pw[:, :64],
                        lhsT=warm_src[:],
                        rhs=warm_src[:, :64],
                        start=(k == 0),
                        stop=(k == 3),
                    )

            for ci, (t0, cs) in enumerate(chunks):
                # h[f, tok] = gelu(sum_d w1[d, f] * x[d, tok] + b1[f])
                h = hpool.tile([P, nF, TOK_CHUNK], bf16, tag="h")
                for fi in range(nF):
                    ph = psum1.tile([P, TOK_CHUNK], f32, tag="ph")
                    for d in range(nD):
                        nc.tensor.matmul(
                            ph[:, :cs],
                            lhsT=w1_tile(fi, d),
                            rhs=x_sb[ci][:, d, :cs],
                            start=(d == 0),
                            stop=(d == nD - 1),
                        )
                    nc.scalar.activation(
                        h[:, fi, :cs],
                        ph[:, :cs],
                        mybir.ActivationFunctionType.Gelu,
                        bias=b1_sb[:, fi : fi + 1],
                    )

                # yT[dout, tok] = sum_f w2[f, dout] * h[f, tok]
                # do-major: each dout's psum completes early so its
                # copy-back + store overlap the next dout's matmuls.
                for do in range(nD):
                    py = psum2.tile([P, TOK_CHUNK], f32, tag="py")
                    for fi in range(nF):
                        nc.tensor.matmul(
                            py[:, :cs],
                            lhsT=w2_tile(fi, do),
                            rhs=h[:, fi, :cs],
                            start=(fi == 0),
                            stop=(fi == nF - 1),
                        )
                    ot = outp.tile([P, TOK_CHUNK], f32, tag="ot")
                    nc.vector.tensor_copy(ot[:, :cs], py[:, :cs])
                    nc.sync.dma_start(
                        yT[do * P : (do + 1) * P, t0 : t0 + cs], ot[:, :cs]
                    )
    nc.compile()
    return nc


def _route(xf, router_w, router_b):
    """Top-2 routing, numpy fp32. Returns (idx1, idx2, g1, g2)."""
    logits = xf @ router_w + router_b
    m = logits.max(axis=-1, keepdims=True)
    p = np.exp(logits - m, dtype=np.float32)
    p /= p.sum(axis=-1, keepdims=True)
    # top-2 indices, ties -> lower index first (matches jax.lax.top_k)
    part = np.argpartition(-p, 1, axis=-1)[:, :2]
    pv = np.take_along_axis(p, part, axis=-1)
    swap = (pv[:, 1] > pv[:, 0]) | ((pv[:, 1] == pv[:, 0]) & (part[:, 1] < part[:, 0]))
    i1 = np.where(swap, part[:, 1], part[:, 0])
    i2 = np.where(swap, part[:, 0], part[:, 1])
    p1 = np.take_along_axis(p, i1[:, None], axis=-1)[:, 0]
    p2 = np.take_along_axis(p, i2[:, None], axis=-1)[:, 0]
    s = p1 + p2
    return i1, i2, p1 / s, p2 / s


def kernel(x, router_w, router_b, w1, b1, w2, b2):
    global last_results
    x = np.asarray(x, dtype=np.float32)
    router_w = np.asarray(router_w, dtype=np.float32)
    router_b = np.asarray(router_b, dtype=np.float32)
    w1 = np.asarray(w1, dtype=np.float32)
    b1 = np.asarray(b1, dtype=np.float32)
    w2 = np.asarray(w2, dtype=np.float32)
    b2 = np.asarray(b2, dtype=np.float32)

    B, S, _ = x.shape
    T = B * S
    xf = x.reshape(T, D)

    i1, i2, g1, g2 = _route(xf, router_w, router_b)

    tok_lists = []
    gate_lists = []
    for e in range(E):
        m1 = i1 == e
        m2 = i2 == e
        toks = np.nonzero(m1 | m2)[0]
        gates = np.where(m1[toks], g1[toks], g2[toks]).astype(np.float32)
        tok_lists.append(toks)
        gate_lists.append(gates)

    # Tokens only ever occupy matmul moving/free dims, so C need not be a
    # multiple of 128 — just 8-aligned (keeps DMA rows word-aligned) and
    # >=512 so the chunking/prefetch structure holds.
    C = max(512, -(-max(len(t) for t in tok_lists) // 8) * 8)
    nD, nF = D // P, F // P

    xf_b = xf.astype(ml_dtypes.bfloat16)
    in_maps = []
    for e in range(E):
        toks = tok_lists[e]
        xT = np.zeros((D, C), dtype=ml_dtypes.bfloat16)
        xT[:, : len(toks)] = xf_b[toks].T
        w1_b = w1[e].astype(ml_dtypes.bfloat16)
        # w1p[fi, p, d*P + c] = w1[d*P + p, fi*P + c]
        w1p = np.ascontiguousarray(
            w1_b.reshape(nD, P, nF, P).transpose(2, 1, 0, 3).reshape(nF, P, nD * P)
        )
        in_maps.append(
            {
                "xT": xT,
                "w1p": w1p,
                "w2": w2[e].astype(ml_dtypes.bfloat16),
                "b1p": np.ascontiguousarray(b1[e].reshape(nF, P).T),
            }
        )

    nc = _build(C)
    trace = bool(int(os.environ.get("KERNEL_TRACE", "0")))
    last_results = run_bass_kernel_spmd(
        nc, in_maps, core_ids=list(range(N_CORES)), trace=trace
    )

    out = np.zeros((T, D), dtype=np.float32)
    for e in range(E):
        toks = tok_lists[e]
        ye = last_results.results[e]["yT"][:, : len(toks)].T
        out[toks] += gate_lists[e][:, None] * (ye + b2[e][None, :])
    return out.reshape(B, S, D)



# revision 6
# speedup vs baseline: 1.1953x; 1.1953x over previous
"""MoE (top-2 of 8 experts) Trainium2 kernel.

Strategy: expert-parallel across 8 NeuronCores, one expert per core.
Host computes the (tiny) router + top-2 token dispatch; each core runs
the heavy expert FFN (x @ w1 -> gelu -> @ w2) over only the tokens
routed to its expert (~T*K/E tokens), in bf16 with fp32 accumulation.
Host applies the renormalized top-2 gates + b2 and scatter-adds the
per-expert outputs back into the full [B,S,D] output.

Shapes (hardcoded from the problem spec): B=4, S=2048, D=768, E=8,
F=4*D=3072, TOP_K=2.
"""

import os
import sys
import types

import numpy as np
import ml_dtypes

# concourse.bass_utils imports antenv.axon_hooks when tracing is requested
# (e.g. BASS_TRACE=1); some deployments lack that module. Provide a stub so
# tracing degrades gracefully (run without trace) instead of crashing.
try:
    from antenv import axon_hooks as _axon_hooks  # noqa: F401
except ImportError:
    _m = types.ModuleType("antenv.axon_hooks")
    _m._hook = None
    _m.set_axon_ntff_profile_hook = lambda h: setattr(_m, "_hook", h)
    _m.get_axon_ntff_profile_hook = lambda: _m._hook
    sys.modules["antenv.axon_hooks"] = _m
    try:
        import antenv

        antenv.axon_hooks = _m
    except ImportError:
        pass

import concourse.bass as bass
import concourse.tile as tile
from concourse import bacc, mybir
from concourse.bass_utils import run_bass_kernel_spmd

P = 128
D = 768
F = 3072
E = 8
TOP_K = 2
N_CORES = 8

bf16 = mybir.dt.bfloat16
f32 = mybir.dt.float32

# Stash of the most recent BassKernelResults (for test harness introspection).
last_results = None


def _chunks_of(total, size):
    """Split into chunks of `size`, avoiding a tail chunk under 256 when
    possible (small-N matmuls pay proportionally more issue overhead)."""
    out = []
    t0 = 0
    while t0 < total:
        rem = total - t0
        if size < rem < size + 256 and rem - 256 >= 256:
            out.append((t0, rem - 256))
            out.append((t0 + rem - 256, 256))
            break
        cs = min(size, rem)
        out.append((t0, cs))
        t0 += cs
    return out


def _build(C):
    """Expert FFN kernel: yT[D, C] = (gelu(x @ w1 + b1) @ w2).T

    Inputs arrive pre-transposed / pre-permuted so every contraction dim
    lands on SBUF partitions without any on-device transpose:
      xT[D, C]          x transposed
      w1p[nF, P, nD*P]  w1p[fi, p, d*P + c] = w1[d*P + p, fi*P + c]
      w2[F, D]          natural layout
      b1p[P, nF]        b1p[p, o] = b1[o*P + p]
    Output is yT[dout, tok] (host transposes back).
    Per-token gates and b2 are intentionally NOT applied here (host does
    that); this keeps every device instruction to <=1 sync wait.
    """
    nD, nF, nT = D // P, F // P, C // P
    nc = bacc.Bacc(
        "TRN2", target_bir_lowering=False, debug=False, num_devices=N_CORES
    )
    xT = nc.declare_dram_parameter("xT", [D, C], bf16, isOutput=False)
    w1p = nc.declare_dram_parameter("w1p", [nF, P, nD * P], bf16, isOutput=False)
    w2 = nc.declare_dram_parameter("w2", [F, D], bf16, isOutput=False)
    b1p = nc.declare_dram_parameter("b1p", [P, nF], f32, isOutput=False)
    yT = nc.declare_dram_parameter("yT", [D, C], bf16, isOutput=True)

    TOK_CHUNK = 512
    chunks = _chunks_of(C, TOK_CHUNK)

    with tile.TileContext(nc) as tc:
        with (
            tc.tile_pool(name="const", bufs=1) as const_pool,
            tc.tile_pool(name="hpool", bufs=2) as hpool,
            tc.tile_pool(name="psum1", bufs=4, space="PSUM") as psum1,
            tc.tile_pool(name="psum2", bufs=3, space="PSUM") as psum2,
            tc.tile_pool(name="outp", bufs=6) as outp,
        ):
            # Input DMA order tuned so the fi=0 compute group can start as
            # early as possible: w1 group 0 (393 KB) first, then chunk-0's
            # x arrives per-d-slice (131 KB each) so MM (fi=0, d) only
            # waits on the slice it consumes, not the whole 786 KB chunk.
            xT_r = xT.rearrange("(o p) t -> p o t", p=P)

            W1G = 2  # f-tiles per w1 DMA group
            w1_sb = []
            t = const_pool.tile([P, W1G, nD * P], bf16, tag="w1g_0")
            nc.sync.dma_start(t[:], w1p[0:W1G].rearrange("f p dc -> p f dc"))
            w1_sb.append(t)

            x_sb = []
            t0, cs = chunks[0]
            xt = const_pool.tile([P, nD, TOK_CHUNK], bf16, tag="x_0")
            for d in range(nD):
                nc.sync.dma_start(
                    xt[:, d, :cs], xT_r[:, d, t0 : t0 + cs]
                )
            x_sb.append(xt)

            for g in range(1, nF // W1G):
                t = const_pool.tile([P, W1G, nD * P], bf16, tag=f"w1g_{g}")
                nc.sync.dma_start(
                    t[:], w1p[g * W1G : (g + 1) * W1G].rearrange("f p dc -> p f dc")
                )
                w1_sb.append(t)
                if g == 1:
                    b1_sb = const_pool.tile([P, nF], f32)
                    nc.sync.dma_start(b1_sb[:], b1p[:, :])
                    # Pre-touch b1 on the scalar engine so the gelu
                    # activations (which carry the bias as a pointer operand
                    # and thus have only one sync-wait slot) never need to
                    # wait on the DMA.
                    scratch = const_pool.tile([P, 1], f32)
                    nc.scalar.copy(scratch[:], b1_sb[:, 0:1])

            W2G = 4  # f-tiles per w2 DMA group
            w2_sb = []
            for g in range(nF // W2G):
                t = const_pool.tile([P, W2G, D], bf16, tag=f"w2g_{g}")
                nc.sync.dma_start(
                    t[:],
                    w2[g * W2G * P : (g + 1) * W2G * P, :].rearrange(
                        "(f p) d -> p f d", p=P
                    ),
                )
                w2_sb.append(t)

            for ci in range(1, len(chunks)):
                t0, cs = chunks[ci]
                xt = const_pool.tile([P, nD, TOK_CHUNK], bf16, tag=f"x_{ci}")
                nc.sync.dma_start(xt[:, :, :cs], xT_r[:, :, t0 : t0 + cs])
                x_sb.append(xt)

            def w1_tile(fi, d):
                return w1_sb[fi // W1G][:, fi % W1G, d * P : (d + 1) * P]

            def w2_tile(fi, do):
                return w2_sb[fi // W2G][:, fi % W2G, do * P : (do + 1) * P]

            # Dummy matmuls on a zeroed tile while input DMAs stream in:
            # keeps the PE busy through the HAM activity window so the
            # real matmuls start at 2.4 GHz instead of the cold 1.2 GHz.
            # 12 groups (~3.6 us) end just as the w1g0 + x-c0-d0 DMA prefix
            # lands; longer warmup would delay the first real matmul.
            warm_src = const_pool.tile([P, P], bf16)
            nc.any.memset(warm_src[:], 0.0)
            for _w in range(12):
                pw = psum1.tile([P, TOK_CHUNK], f32, tag="ph", name="pw")
                for k in range(4):
                    nc.tensor.matmul(
                        pw[:, :64],
                        lhsT=warm_src[:],
                        rhs=warm_src[:, :64],
                        start=(k == 0),
                        stop=(k == 3),
                    )

            for ci, (t0, cs) in enumerate(chunks):
                # h[f, tok] = gelu(sum_d w1[d, f] * x[d, tok] + b1[f])
                h = hpool.tile([P, nF, TOK_CHUNK], bf16, tag="h")
                for fi in range(nF):
                    ph = psum1.tile([P, TOK_CHUNK], f32, tag="ph")
                    for d in range(nD):
                        nc.tensor.matmul(
                            ph[:, :cs],
                            lhsT=w1_tile(fi, d),
                            rhs=x_sb[ci][:, d, :cs],
                            start=(d == 0),
                            stop=(d == nD - 1),
                        )
                    nc.scalar.activation(
                        h[:, fi, :cs],
                        ph[:, :cs],
                        mybir.ActivationFunctionType.Gelu,
                        bias=b1_sb[:, fi : fi + 1],
                    )

                # yT[dout, tok] = sum_f w2[f, dout] * h[f, tok]
                # do-major: each dout's psum completes early so its
                # copy-back + store overlap the next dout's matmuls.
                for do in range(nD):
                    py = psum2.tile([P, TOK_CHUNK], f32, tag="py")
                    for fi in range(nF):
                        nc.tensor.matmul(
                            py[:, :cs],
                            lhsT=w2_tile(fi, do),
                            rhs=h[:, fi, :cs],
                            start=(fi == 0),
                            stop=(fi == nF - 1),
                        )
                    # bf16 output staging: halves the output DMA bytes (the
                    # last chunk's store is the pre-epilogue tail) at ~0.1%
                    # extra error, well within the 2e-2 budget.
                    ot = outp.tile([P, TOK_CHUNK], bf16, tag="ot")
                    nc.vector.tensor_copy(ot[:, :cs], py[:, :cs])
                    nc.sync.dma_start(
                        yT[do * P : (do + 1) * P, t0 : t0 + cs], ot[:, :cs]
                    )
    nc.compile()
    return nc


def _route(xf, router_w, router_b):
    """Top-2 routing, numpy fp32. Returns (idx1, idx2, g1, g2)."""
    logits = xf @ router_w + router_b
    m = logits.max(axis=-1, keepdims=True)
    p = np.exp(logits - m, dtype=np.float32)
    p /= p.sum(axis=-1, keepdims=True)
    # top-2 indices, ties -> lower index first (matches jax.lax.top_k)
    part = np.argpartition(-p, 1, axis=-1)[:, :2]
    pv = np.take_along_axis(p, part, axis=-1)
    swap = (pv[:, 1] > pv[:, 0]) | ((pv[:, 1] == pv[:, 0]) & (part[:, 1] < part[:, 0]))
    i1 = np.where(swap, part[:, 1], part[:, 0])
    i2 = np.where(swap, part[:, 0], part[:, 1])
    p1 = np.take_along_axis(p, i1[:, None], axis=-1)[:, 0]
    p2 = np.take_along_axis(p, i2[:, None], axis=-1)[:, 0]
    s = p1 + p2
    return i1, i2, p1 / s, p2 / s


def kernel(x, router_w, router_b, w1, b1, w2, b2):
    global last_results
    x = np.asarray(x, dtype=np.float32)
    router_w = np.asarray(router_w, dtype=np.float32)
    router_b = np.asarray(router_b, dtype=np.float32)
    w1 = np.asarray(w1, dtype=np.float32)
    b1 = np.asarray(b1, dtype=np.float32)
    w2 = np.asarray(w2, dtype=np.float32)
    b2 = np.asarray(b2, dtype=np.float32)

    B, S, _ = x.shape
    T = B * S
    xf = x.reshape(T, D)

    i1, i2, g1, g2 = _route(xf, router_w, router_b)

    tok_lists = []
    gate_lists = []
    for e in range(E):
        m1 = i1 == e
        m2 = i2 == e
        toks = np.nonzero(m1 | m2)[0]
        gates = np.where(m1[toks], g1[toks], g2[toks]).astype(np.float32)
        tok_lists.append(toks)
        gate_lists.append(gates)

    # Tokens only ever occupy matmul moving/free dims, so C need not be a
    # multiple of 128 — just 8-aligned (keeps DMA rows word-aligned) and
    # >=512 so the chunking/prefetch structure holds.
    C = max(512, -(-max(len(t) for t in tok_lists) // 8) * 8)
    nD, nF = D // P, F // P

    xf_b = xf.astype(ml_dtypes.bfloat16)
    in_maps = []
    for e in range(E):
        toks = tok_lists[e]
        xT = np.zeros((D, C), dtype=ml_dtypes.bfloat16)
        xT[:, : len(toks)] = xf_b[toks].T
        w1_b = w1[e].astype(ml_dtypes.bfloat16)
        # w1p[fi, p, d*P + c] = w1[d*P + p, fi*P + c]
        w1p = np.ascontiguousarray(
            w1_b.reshape(nD, P, nF, P).transpose(2, 1, 0, 3).reshape(nF, P, nD * P)
        )
        in_maps.append(
            {
                "xT": xT,
                "w1p": w1p,
                "w2": w2[e].astype(ml_dtypes.bfloat16),
                "b1p": np.ascontiguousarray(b1[e].reshape(nF, P).T),
            }
        )

    nc = _build(C)
    trace = bool(int(os.environ.get("KERNEL_TRACE", "0")))
    last_results = run_bass_kernel_spmd(
        nc, in_maps, core_ids=list(range(N_CORES)), trace=trace
    )

    out = np.zeros((T, D), dtype=np.float32)
    for e in range(E):
        toks = tok_lists[e]
        ye = last_results.results[e]["yT"][:, : len(toks)].T.astype(np.float32)
        out[toks] += gate_lists[e][:, None] * (ye + b2[e][None, :])
    return out.reshape(B, S, D)

